# revision 1
# baseline (speedup 1.0000x reference)
"""Trainium2 Bass kernel for a full attention layer (QKV proj + interleaved
RoPE + non-causal SDPA + output proj), tensor-parallel over heads on 8
NeuronCores.

Hardcoded problem shape: B=2, S=2048, HID=2048, H=16 heads, DH=128, fp32.

Sharding (per core c of 8): heads 2c, 2c+1.
 - w_qkv rows for those heads (q/k rows de-interleaved per head so RoPE's
   (2i, 2i+1) pairing becomes a 64-partition block swap), transposed to
   [HID, 256] so the contraction dim (HID) rides the SBUF partition axis.
 - w_o columns for those heads, transposed to [256, HID].
 - hidden_states transposed to [HID, B*S] (replicated to every core).
 - cos/sin prepped as de-interleaved, transposed [128, S] tiles; sin carries
   the rotate-half sign in its first 64 rows.
Each core computes a full-shape partial output [B*S, HID] (its heads'
contribution through w_o); the host unshards by summing the 8 partials.

All matmuls run as float32r (full PE rate for moving dim >= 256; fp32 data).
Attention is computed in the S^T orientation: scores come out as
P^T[k, q] tiles so the AV matmul can contract k on the partition axis with
no transposes anywhere.  The softmax denominator is an all-ones [128,128]
stationary matmul, which lands sum_k P broadcast across all 128 partitions
for free; out tiles are scaled by its reciprocal after AV (divide-after-AV).
exp() is fused into the PSUM->SBUF drain on the scalar engine with the
1/sqrt(DH) scale folded in.  No max-subtraction: scores are ~N(0,1) so
exp is safe in fp32.
"""

import os

import numpy as np

B, S, HID = 2, 2048, 2048
H, DH = 16, 128
NC = 8
HPC = H // NC          # heads per core = 2
OC = HPC * DH          # per-core o width per section = 256
T = B * S              # 4096 tokens
KT = HID // 128        # 16 contraction tiles
TC = 256               # token chunk for QKV projection
QC = 512               # query chunk for attention
SCALE = 1.0 / float(np.sqrt(DH))

_exec_time_ns = None   # stashed by kernel() for the test harness


def _build(reps=1):
    import concourse.bacc as bacc
    import concourse.mybir as mybir
    import concourse.tile as tile

    f32 = mybir.dt.float32
    fr = mybir.dt.float32r
    Exp = mybir.ActivationFunctionType.Exp

    def r(ap):
        return ap

    nc = bacc.Bacc("TRN2", target_bir_lowering=False)

    hT = nc.dram_tensor("hT", [HID, T], fr, kind="ExternalInput")
    wqT = nc.dram_tensor("wqT", [HID, OC], fr, kind="ExternalInput")
    wkT = nc.dram_tensor("wkT", [HID, OC], fr, kind="ExternalInput")
    wvT = nc.dram_tensor("wvT", [HID, OC], fr, kind="ExternalInput")
    woT = nc.dram_tensor("woT", [OC, HID], fr, kind="ExternalInput")
    cc = nc.dram_tensor("cc", [DH, S], f32, kind="ExternalInput")
    ss = nc.dram_tensor("ss", [DH, S], f32, kind="ExternalInput")
    out_p = nc.dram_tensor("out_p", [T, HID], f32, kind="ExternalOutput")

    hT_r = hT.rearrange("(k p) t -> p k t", p=128)      # [128, 16, T]

    with tile.TileContext(nc) as tc:
        with (
            tc.tile_pool(name="const", bufs=1) as constp,
            tc.tile_pool(name="hbuf", bufs=2) as hpool,
            tc.tile_pool(name="qkv", bufs=1) as qkvp,
            tc.tile_pool(name="rope", bufs=2) as ropep,
            tc.tile_pool(name="pbuf", bufs=6) as pp,
            tc.tile_pool(name="small", bufs=2) as smallp,
            tc.tile_pool(name="fout", bufs=4) as foutp,
        ):
            # ---- resident weights/constants (per-ktile tiles: 1 DMA -> 1 sem) ----
            # (re-emitted per rep for benchmarking; tags make slots reuse)
            for _rep in range(reps):
             wqT_r = wqT.rearrange("(k p) o -> p k o", p=128)
             wkT_r = wkT.rearrange("(k p) o -> p k o", p=128)
             wvT_r = wvT.rearrange("(k p) o -> p k o", p=128)
             woT_r = woT.rearrange("(h p) n -> p h n", p=128)
             wq_t, wk_t, wv_t = [], [], []
             for kk in range(KT):
                 for lst, srcr, nm in (
                     (wq_t, wqT_r, "wq"),
                     (wk_t, wkT_r, "wk"),
                     (wv_t, wvT_r, "wv"),
                 ):
                     t = constp.tile([128, OC], fr, tag=f"{nm}{kk}")
                     nc.sync.dma_start(out=t, in_=srcr[:, kk, :])
                     lst.append(t)
             wo_t = []
             for hl in range(HPC):
                 t = constp.tile([128, HID], fr, tag=f"wo{hl}")
                 nc.sync.dma_start(out=t, in_=woT_r[:, hl, :])
                 wo_t.append(t)
             cc_sb = constp.tile([128, S], f32)
             ss_sb = constp.tile([128, S], f32)
             nc.sync.dma_start(out=cc_sb, in_=cc[:, :])
             nc.sync.dma_start(out=ss_sb, in_=ss[:, :])
             ones_f32 = constp.tile([128, 128], f32)
             nc.vector.memset(ones_f32, 1.0)
             ones_sb = constp.tile([128, 128], fr)
             nc.scalar.copy(ones_sb, ones_f32)

             for b in range(B):
                 t0 = b * S

                 # ---- phase 1: QKV projection (+ fused RoPE for q,k) ----
                 # qk_sb rows: [q_h0, q_h1, k_h0, k_h1], each [128 d, S]
                 qk_sb = qkvp.tile([128, 4, S], fr, tag="qk")
                 v_sb = qkvp.tile([128, S // 128, OC], fr, tag="v")
                 w_of = [(wq_t, 0), (wq_t, 1), (wk_t, 0), (wk_t, 1)]
                 with tc.tile_pool(name="ps1", bufs=2, space="PSUM") as ps1:
                     for tci in range(S // TC):
                         soff = tci * TC
                         hch = []
                         for kk in range(KT):
                             ht = hpool.tile([128, TC], fr, tag=f"hch{kk}")
                             nc.sync.dma_start(
                                 out=ht, in_=hT_r[:, kk, t0 + soff : t0 + soff + TC]
                             )
                             hch.append(ht)
                         for ot in range(4):
                             wsb, hl = w_of[ot]
                             ps = ps1.tile([128, TC], f32, tag="ps_qk")
                             for kk in range(KT):
                                 nc.tensor.matmul(
                                     ps,
                                     r(wsb[kk][:, hl * DH : (hl + 1) * DH]),
                                     r(hch[kk]),
                                     start=(kk == 0),
                                     stop=(kk == KT - 1),
                                 )
                             # RoPE: dst = raw*cc + blockswap(raw)*ss_signed
                             raw = ropep.tile([128, TC], f32, tag="raw")
                             nc.scalar.copy(raw, ps)
                             swp = ropep.tile([128, TC], f32, tag="swp")
                             nc.sync.dma_start(out=swp[0:64, :], in_=raw[64:128, :])
                             nc.sync.dma_start(out=swp[64:128, :], in_=raw[0:64, :])
                             tmp = ropep.tile([128, TC], f32, tag="tmp")
                             nc.vector.tensor_mul(tmp, raw, cc_sb[:, soff : soff + TC])
                             nc.vector.tensor_mul(swp, swp, ss_sb[:, soff : soff + TC])
                             nc.vector.tensor_add(
                                 qk_sb[:, ot, soff : soff + TC], tmp, swp
                             )
                         for tt in range(TC // 128):
                             psv = ps1.tile([128, OC], f32, tag="ps_v")
                             for kk in range(KT):
                                 nc.tensor.matmul(
                                     psv,
                                     r(hch[kk][:, tt * 128 : (tt + 1) * 128]),
                                     r(wv_t[kk]),
                                     start=(kk == 0),
                                     stop=(kk == KT - 1),
                                 )
                             nc.scalar.copy(v_sb[:, tci * (TC // 128) + tt, :], psv)

                 # ---- phase 2: attention per head ----
                 outT_sb = qkvp.tile([128, HPC, S], fr, tag="outT")
                 with (
                     tc.tile_pool(name="ps2s", bufs=4, space="PSUM") as ps2s,
                     tc.tile_pool(name="ps2od", bufs=1, space="PSUM") as ps2od,
                 ):
                     for hl in range(HPC):
                         qTap = qk_sb[:, hl, :]
                         kTap = qk_sb[:, 2 + hl, :]
                         for qci in range(S // QC):
                             q0 = qci * QC
                             psO = ps2od.tile([128, QC], f32, tag="psO")
                             psD = ps2od.tile([128, QC], f32, tag="psD")
                             nkt = S // 128
                             for kg in range(nkt // 4):
                                 pexp = []
                                 for j in range(4):
                                     kt = kg * 4 + j
                                     pss = ps2s.tile([128, QC], f32, tag="pss")
                                     nc.tensor.matmul(
                                         pss,
                                         r(kTap[:, kt * 128 : (kt + 1) * 128]),
                                         r(qTap[:, q0 : q0 + QC]),
                                         skip_group_check=True,
                                     )
                                     pe = pp.tile([128, QC], fr, tag="pexp")
                                     nc.scalar.activation(pe, pss, Exp, scale=SCALE)
                                     pexp.append(pe)
                                 for j in range(4):
                                     kt = kg * 4 + j
                                     first = kt == 0
                                     last = kt == nkt - 1
                                     nc.tensor.matmul(
                                         psO,
                                         r(v_sb[:, kt, hl * DH : (hl + 1) * DH]),
                                         r(pexp[j]),
                                         start=first,
                                         stop=last,
                                         skip_group_check=True,
                                     )
                                     nc.tensor.matmul(
                                         psD,
                                         r(ones_sb),
                                         r(pexp[j]),
                                         start=first,
                                         stop=last,
                                         skip_group_check=True,
                                     )
                             rd = smallp.tile([128, QC], f32, tag="rd")
                             nc.vector.reciprocal(rd, psD)
                             nc.vector.tensor_mul(
                                 outT_sb[:, hl, q0 : q0 + QC], psO, rd
                             )

                 # ---- phase 3: output projection (partial over this core's heads) ----
                 with tc.tile_pool(name="ps3", bufs=4, space="PSUM") as ps3:
                     for tt in range(S // 128):
                         for nh in range(HID // 512):
                             psF = ps3.tile([128, 512], f32, tag="psF")
                             for hl in range(HPC):
                                 nc.tensor.matmul(
                                     psF,
                                     r(outT_sb[:, hl, tt * 128 : (tt + 1) * 128]),
                                     r(wo_t[hl][:, nh * 512 : (nh + 1) * 512]),
                                     start=(hl == 0),
                                     stop=(hl == HPC - 1),
                                 )
                             fo = foutp.tile([128, 512], f32, tag="fo")
                             nc.scalar.copy(fo, psF)
                             nc.sync.dma_start(
                                 out=out_p[
                                     t0 + tt * 128 : t0 + (tt + 1) * 128,
                                     nh * 512 : (nh + 1) * 512,
                                 ],
                                 in_=fo,
                             )

    nc.compile()
    return nc


def _deint(idx128):
    """de-interleave a [128] index block: evens then odds."""
    return np.concatenate([idx128[0::2], idx128[1::2]])


def _prep_inputs(hidden_states, cos, sin, w_qkv, w_o):
    """Host-side shard/layout prep. Returns per-core input maps."""
    hs = np.ascontiguousarray(
        hidden_states.reshape(T, HID).T, dtype=np.float32
    )  # [HID, T]
    ccf = np.ascontiguousarray(
        np.concatenate([cos.T[0::2, :], cos.T[1::2, :]], axis=0), dtype=np.float32
    )  # [128, S] de-interleaved
    ssf = np.ascontiguousarray(
        np.concatenate([-sin.T[0::2, :], sin.T[1::2, :]], axis=0), dtype=np.float32
    )  # [128, S] de-interleaved, sign folded

    in_maps = []
    for c in range(NC):
        heads = [HPC * c + i for i in range(HPC)]
        qrows = np.concatenate([_deint(np.arange(h * DH, (h + 1) * DH)) for h in heads])
        krows = H * DH + qrows
        vrows = (
            np.concatenate([np.arange(h * DH, (h + 1) * DH) for h in heads])
            + 2 * H * DH
        )
        ocols = np.concatenate([np.arange(h * DH, (h + 1) * DH) for h in heads])
        in_maps.append(
            {
                "hT": hs,
                "wqT": np.ascontiguousarray(w_qkv[qrows, :].T, dtype=np.float32),
                "wkT": np.ascontiguousarray(w_qkv[krows, :].T, dtype=np.float32),
                "wvT": np.ascontiguousarray(w_qkv[vrows, :].T, dtype=np.float32),
                "woT": np.ascontiguousarray(w_o[:, ocols].T, dtype=np.float32),
                "cc": ccf,
                "ss": ssf,
            }
        )
    return in_maps


def kernel(hidden_states, cos, sin, w_qkv, w_o):
    global _exec_time_ns
    from concourse.bass_utils import run_bass_kernel_spmd

    hidden_states = np.asarray(hidden_states, dtype=np.float32)
    cos = np.asarray(cos, dtype=np.float32)
    sin = np.asarray(sin, dtype=np.float32)
    w_qkv = np.asarray(w_qkv, dtype=np.float32)
    w_o = np.asarray(w_o, dtype=np.float32)

    nc = _build()
    in_maps = _prep_inputs(hidden_states, cos, sin, w_qkv, w_o)
    res = run_bass_kernel_spmd(
        nc,
        in_maps,
        core_ids=list(range(NC)),
        trace=bool(int(os.environ.get("KERNEL_TRACE", "0"))),
    )
    _exec_time_ns = res.exec_time_ns

    acc = res.results[0]["out_p"].astype(np.float32).copy()
    for c in range(1, NC):
        acc += res.results[c]["out_p"]
    return acc.reshape(B, S, HID)



# revision 2
# speedup vs baseline: 1.8406x; 1.8406x over previous
"""Trainium2 Bass kernel for a full attention layer (QKV proj + interleaved
RoPE + non-causal SDPA + output proj) on 8 NeuronCores.

Hardcoded problem shape: B=2, S=2048, HID=2048, H=16 heads, DH=128, fp32 I/O.

Sharding: batch-parallel x head-parallel. Core c handles batch c//4 and the
4 heads [4*(c%4), 4*(c%4)+4). Each core computes a full-shape partial output
[S, HID] (its heads' contribution through w_o); the host unshards by summing
the 4 partials per batch.

All matmuls run in bf16 (fp32 PSUM accumulation): same 1 col/cycle PE rate as
float32r but FWL halves the weight-load shadow, DMA bytes halve, and DVE gets
its 2x packed mode. Error budget is fine for the 2e-2 gate (bf16 rounding is
~0.4% per tensor, independent roundings wash out in the K=2048 contractions).

Layouts (host-prepped): weights transposed so contraction (HID) rides the
partition axis; q/k rows de-interleaved per head so RoPE's (2i,2i+1) pairing
becomes a 64-partition block swap, done with cross-partition-base DVE
multiplies (no DMA, no extra copies): sin is host-swapped so both DVE inputs
share a partition base and only the output base is shifted.

Attention runs in the S^T orientation (scores come out as P^T[k,q]) so AV
contracts k on the partition axis with no transposes. exp is fused into the
PSUM->SBUF drain on the scalar engine over 1024-wide 2-bank PSUM tiles (two
score matmuls per exp). The softmax denominator is an all-ones stationary
matmul over pair-added P tiles (half the PE columns); its reciprocal uses the
fast custom-DVE op, and out tiles are scaled after AV (divide-after-AV).
No max-subtraction: scores are ~N(0,1) so exp is safe.
"""

import os

import numpy as np
import ml_dtypes

BF = ml_dtypes.bfloat16

B, S, HID = 2, 2048, 2048
H, DH = 16, 128
NC = 8
GPB = 4                # cores per batch group
HPC = H // GPB         # heads per core = 4
OC = HPC * DH          # per-core qkv width per section = 512
KT = HID // 128        # 16 contraction tiles
CH = 512               # token chunk for QKV projection
QC = 512               # query chunk for attention
NKB = S // 128         # 16 key blocks
SCALE = 1.0 / float(np.sqrt(DH))

_exec_time_ns = None   # stashed by kernel() for the test harness


def _build():
    import concourse.bacc as bacc
    import concourse.mybir as mybir
    import concourse.tile as tile

    f32 = mybir.dt.float32
    b16 = mybir.dt.bfloat16
    Exp = mybir.ActivationFunctionType.Exp

    nc = bacc.Bacc("TRN2", target_bir_lowering=False)

    hT = nc.dram_tensor("hT", [HID, S], b16, kind="ExternalInput")
    wqT = nc.dram_tensor("wqT", [HID, OC], b16, kind="ExternalInput")
    wkT = nc.dram_tensor("wkT", [HID, OC], b16, kind="ExternalInput")
    wvT = nc.dram_tensor("wvT", [HID, OC], b16, kind="ExternalInput")
    woT = nc.dram_tensor("woT", [OC, HID], b16, kind="ExternalInput")
    cc = nc.dram_tensor("cc", [DH, S], b16, kind="ExternalInput")
    ssw = nc.dram_tensor("ssw", [DH, S], b16, kind="ExternalInput")
    ones = nc.dram_tensor("ones", [128, 128], b16, kind="ExternalInput")
    out_p = nc.dram_tensor("out_p", [S, HID], f32, kind="ExternalOutput")

    hT_r = hT.rearrange("(k p) t -> p k t", p=128)       # [128, 16, S]
    wqT_r = wqT.rearrange("(k p) o -> p k o", p=128)     # [128, 16, OC]
    wkT_r = wkT.rearrange("(k p) o -> p k o", p=128)
    wvT_r = wvT.rearrange("(k p) o -> p k o", p=128)
    woT_r = woT.rearrange("(h p) n -> p h n", p=128)     # [128, 4, HID]

    with tile.TileContext(nc) as tc:
        with (
            tc.tile_pool(name="const", bufs=1) as constp,
            tc.tile_pool(name="qkv", bufs=1) as qkvp,
            tc.tile_pool(name="rope", bufs=3) as ropep,
            tc.tile_pool(name="pbuf", bufs=3) as pp,
            tc.tile_pool(name="small", bufs=2) as smallp,
        ):
            # ---- resident weights/constants ----
            wq_sb = constp.tile([128, KT, OC], b16, tag="wq")
            wk_sb = constp.tile([128, KT, OC], b16, tag="wk")
            wv_sb = constp.tile([128, KT, OC], b16, tag="wv")
            for dst, src in ((wq_sb, wqT_r), (wk_sb, wkT_r), (wv_sb, wvT_r)):
                for kg in range(4):
                    nc.sync.dma_start(
                        out=dst[:, kg * 4 : (kg + 1) * 4, :],
                        in_=src[:, kg * 4 : (kg + 1) * 4, :],
                    )
            wo_sb = constp.tile([128, HPC, HID], b16, tag="wo")
            for hl in range(HPC):
                nc.sync.dma_start(out=wo_sb[:, hl, :], in_=woT_r[:, hl, :])
            cc_sb = constp.tile([128, S], b16, tag="cc")
            ssw_sb = constp.tile([128, S], b16, tag="ssw")
            nc.sync.dma_start(out=cc_sb, in_=cc[:, :])
            nc.sync.dma_start(out=ssw_sb, in_=ssw[:, :])
            ones_sb = constp.tile([128, 128], b16, tag="ones")
            nc.sync.dma_start(out=ones_sb, in_=ones[:, :])

            qT_sb = qkvp.tile([128, HPC, S], b16, tag="qT")
            kT_sb = qkvp.tile([128, HPC, S], b16, tag="kT")
            v_sb = qkvp.tile([128, NKB, OC], b16, tag="v")

            # ---- phase 1: QKV projection (+ fused RoPE for q,k) ----
            with (
                tc.tile_pool(name="hbuf", bufs=2) as hpool,
                tc.tile_pool(name="ps1qk", bufs=4, space="PSUM") as ps1qk,
                tc.tile_pool(name="ps1v", bufs=2, space="PSUM") as ps1v,
            ):
                for ci in range(S // CH):
                    soff = ci * CH
                    hch = []
                    for kk in range(KT):
                        ht = hpool.tile([128, CH], b16, tag=f"hch{kk}")
                        nc.sync.dma_start(
                            out=ht, in_=hT_r[:, kk, soff : soff + CH]
                        )
                        hch.append(ht)
                    # 8 outputs: q then k for each of the 4 heads
                    for qk, (wsb, dst) in enumerate(
                        ((wq_sb, qT_sb), (wk_sb, kT_sb))
                    ):
                        for hl in range(HPC):
                            ps = ps1qk.tile([128, CH], f32, tag="ps_qk")
                            for kk in range(KT):
                                nc.tensor.matmul(
                                    ps,
                                    wsb[:, kk, hl * DH : (hl + 1) * DH],
                                    hch[kk],
                                    start=(kk == 0),
                                    stop=(kk == KT - 1),
                                )
                            # RoPE: out = raw*cc + blockswap(raw)*ssw_signed
                            raw = ropep.tile([128, CH], b16, tag="raw")
                            nc.scalar.copy(raw, ps)
                            tmp = ropep.tile([128, CH], b16, tag="tmp")
                            nc.vector.tensor_mul(
                                tmp[0:64, :],
                                raw[64:128, :],
                                ssw_sb[64:128, soff : soff + CH],
                            )
                            nc.vector.tensor_mul(
                                tmp[64:128, :],
                                raw[0:64, :],
                                ssw_sb[0:64, soff : soff + CH],
                            )
                            dslice = dst[:, hl, soff : soff + CH]
                            nc.vector.tensor_mul(
                                dslice, raw, cc_sb[:, soff : soff + CH]
                            )
                            nc.vector.tensor_add(dslice, dslice, tmp)
                    for tt in range(CH // 128):
                        psv = ps1v.tile([128, OC], f32, tag="ps_v")
                        for kk in range(KT):
                            nc.tensor.matmul(
                                psv,
                                hch[kk][:, tt * 128 : (tt + 1) * 128],
                                wv_sb[:, kk, :],
                                start=(kk == 0),
                                stop=(kk == KT - 1),
                            )
                        nc.scalar.copy(v_sb[:, ci * (CH // 128) + tt, :], psv)

            # ---- phase 2: attention per head ----
            outT_sb = qkvp.tile([128, HPC, S], b16, tag="outT")
            with (
                tc.tile_pool(name="ps2s", bufs=2, space="PSUM") as ps2s,
                tc.tile_pool(name="ps2od", bufs=2, space="PSUM") as ps2od,
            ):
                for hl in range(HPC):
                    for qci in range(S // QC):
                        q0 = qci * QC
                        qmv = qT_sb[:, hl, q0 : q0 + QC]
                        psO = ps2od.tile([128, QC], f32, tag="psO")
                        psD = ps2od.tile([128, QC], f32, tag="psD")
                        for kg in range(NKB // 2):
                            pss = ps2s.tile([128, 2 * QC], f32, tag="pss")
                            for j in range(2):
                                kt = kg * 2 + j
                                nc.tensor.matmul(
                                    pss[:, j * QC : (j + 1) * QC],
                                    kT_sb[:, hl, kt * 128 : (kt + 1) * 128],
                                    qmv,
                                    skip_group_check=True,
                                )
                            pe = pp.tile([128, 2 * QC], b16, tag="pexp")
                            nc.scalar.activation(pe, pss, Exp, scale=SCALE)
                            for j in range(2):
                                kt = kg * 2 + j
                                nc.tensor.matmul(
                                    psO,
                                    v_sb[:, kt, hl * DH : (hl + 1) * DH],
                                    pe[:, j * QC : (j + 1) * QC],
                                    start=(kt == 0),
                                    stop=(kt == NKB - 1),
                                    skip_group_check=True,
                                )
                            padd = pp.tile([128, QC], b16, tag="padd")
                            nc.vector.tensor_add(
                                padd, pe[:, 0:QC], pe[:, QC : 2 * QC]
                            )
                            nc.tensor.matmul(
                                psD,
                                ones_sb,
                                padd,
                                start=(kg == 0),
                                stop=(kg == NKB // 2 - 1),
                                skip_group_check=True,
                            )
                        dsb = smallp.tile([128, QC], f32, tag="dsb")
                        nc.vector.tensor_copy(dsb, psD)
                        rd = smallp.tile([128, QC], f32, tag="rd")
                        nc.vector.reciprocal_approx_fast(out=rd, in_=dsb)
                        nc.vector.tensor_mul(
                            outT_sb[:, hl, q0 : q0 + QC], psO, rd
                        )

            # ---- phase 3: output projection (partial over this core's heads) ----
            with (
                tc.tile_pool(name="fout", bufs=4) as foutp,
                tc.tile_pool(name="ps3", bufs=4, space="PSUM") as ps3,
            ):
                for tt in range(S // 128):
                    for nh in range(HID // 512):
                        psF = ps3.tile([128, 512], f32, tag="psF")
                        for hl in range(HPC):
                            nc.tensor.matmul(
                                psF,
                                outT_sb[:, hl, tt * 128 : (tt + 1) * 128],
                                wo_sb[:, hl, nh * 512 : (nh + 1) * 512],
                                start=(hl == 0),
                                stop=(hl == HPC - 1),
                            )
                        fo = foutp.tile([128, 512], f32, tag="fo")
                        if nh % 2 == 0:
                            nc.scalar.copy(fo, psF)
                        else:
                            nc.vector.tensor_copy(fo, psF)
                        nc.sync.dma_start(
                            out=out_p[
                                tt * 128 : (tt + 1) * 128,
                                nh * 512 : (nh + 1) * 512,
                            ],
                            in_=fo,
                        )

    nc.compile()
    return nc


def _deint(idx128):
    """de-interleave a [128] index block: evens then odds."""
    return np.concatenate([idx128[0::2], idx128[1::2]])


def _prep_inputs(hidden_states, cos, sin, w_qkv, w_o):
    """Host-side shard/layout prep. Returns per-core input maps."""
    # cos/sin transposed, de-interleaved: rows 0:64 = dims 0,2,..126 and
    # 64:128 = dims 1,3,..127. cos rows are pairwise equal so both halves
    # match. ssw is the sign-folded sin, pre-block-swapped so the RoPE
    # cross-partition multiplies read input partitions at one base:
    #   out[0:64]  = raw[64:128] * ssw[64:128]   (= -sin * odd part)
    #   out[64:128]= raw[0:64]   * ssw[0:64]     (= +sin * even part)
    ccf = np.concatenate([cos.T[0::2, :], cos.T[1::2, :]], axis=0).astype(BF)
    ssf = np.concatenate([sin.T[1::2, :], -sin.T[0::2, :]], axis=0).astype(BF)
    ones = np.ones((128, 128), dtype=BF)

    hT_b = [
        np.ascontiguousarray(hidden_states[b].T).astype(BF) for b in range(B)
    ]

    in_maps = []
    for c in range(NC):
        b = c // GPB
        heads = [HPC * (c % GPB) + i for i in range(HPC)]
        qrows = np.concatenate([_deint(np.arange(h * DH, (h + 1) * DH)) for h in heads])
        krows = H * DH + qrows
        vrows = (
            np.concatenate([np.arange(h * DH, (h + 1) * DH) for h in heads])
            + 2 * H * DH
        )
        ocols = np.concatenate([np.arange(h * DH, (h + 1) * DH) for h in heads])
        in_maps.append(
            {
                "hT": hT_b[b],
                "wqT": np.ascontiguousarray(w_qkv[qrows, :].T).astype(BF),
                "wkT": np.ascontiguousarray(w_qkv[krows, :].T).astype(BF),
                "wvT": np.ascontiguousarray(w_qkv[vrows, :].T).astype(BF),
                "woT": np.ascontiguousarray(w_o[:, ocols].T).astype(BF),
                "cc": ccf,
                "ssw": ssf,
                "ones": ones,
            }
        )
    return in_maps


def kernel(hidden_states, cos, sin, w_qkv, w_o):
    global _exec_time_ns
    from concourse.bass_utils import run_bass_kernel_spmd

    hidden_states = np.asarray(hidden_states, dtype=np.float32)
    cos = np.asarray(cos, dtype=np.float32)
    sin = np.asarray(sin, dtype=np.float32)
    w_qkv = np.asarray(w_qkv, dtype=np.float32)
    w_o = np.asarray(w_o, dtype=np.float32)

    nc = _build()
    in_maps = _prep_inputs(hidden_states, cos, sin, w_qkv, w_o)
    res = run_bass_kernel_spmd(
        nc,
        in_maps,
        core_ids=list(range(NC)),
        trace=bool(int(os.environ.get("KERNEL_TRACE", "0"))),
    )
    _exec_time_ns = res.exec_time_ns

    out = np.empty((B, S, HID), dtype=np.float32)
    for b in range(B):
        acc = res.results[b * GPB]["out_p"].astype(np.float32).copy()
        for c in range(b * GPB + 1, (b + 1) * GPB):
            acc += res.results[c]["out_p"]
        out[b] = acc
    return out


# revision 5
# speedup vs baseline: 1.9528x; 1.0610x over previous
"""Trainium2 Bass kernel for a full attention layer (QKV proj + interleaved
RoPE + non-causal SDPA + output proj) on 8 NeuronCores.

Hardcoded problem shape: B=2, S=2048, HID=2048, H=16 heads, DH=128, fp32 I/O.

Sharding: batch-parallel x head-parallel. Core c handles batch c//4 and the
4 heads [4*(c%4), 4*(c%4)+4). Each core computes a full-shape partial output
[S, HID] (its heads' contribution through w_o); the host unshards by summing
the 4 partials per batch.

All matmuls run in bf16 (fp32 PSUM accumulation): same 1 col/cycle PE rate as
float32r but FWL halves the weight-load shadow, DMA bytes halve, and DVE gets
its 2x packed mode. Error budget is fine for the 2e-2 gate (bf16 rounding is
~0.4% per tensor, independent roundings wash out in the K=2048 contractions).

Layouts (host-prepped): weights transposed so contraction (HID) rides the
partition axis; q/k rows de-interleaved per head so RoPE's (2i,2i+1) pairing
becomes a 64-partition block swap, done with cross-partition-base DVE
multiplies (no DMA, no extra copies): sin is host-swapped so both DVE inputs
share a partition base and only the output base is shifted.

Attention runs in the S^T orientation (scores come out as P^T[k,q]) so AV
contracts k on the partition axis with no transposes. exp is fused into the
PSUM->SBUF drain on the scalar engine over 1024-wide 2-bank PSUM tiles (two
score matmuls per exp). The softmax denominator is an all-ones stationary
matmul over pair-added P tiles (half the PE columns); its reciprocal uses the
fast custom-DVE op, and out tiles are scaled after AV (divide-after-AV).
No max-subtraction: scores are ~N(0,1) so exp is safe.
"""

import os

import numpy as np
import ml_dtypes

BF = ml_dtypes.bfloat16

B, S, HID = 2, 2048, 2048
H, DH = 16, 128
NC = 8
GPB = 4                # cores per batch group
HPC = H // GPB         # heads per core = 4
OC = HPC * DH          # per-core qkv width per section = 512
KT = HID // 128        # 16 contraction tiles
CH = 512               # token chunk for QKV projection
QC = 512               # query chunk for attention
NKB = S // 128         # 16 key blocks
SCALE = 1.0 / float(np.sqrt(DH))

_exec_time_ns = None   # stashed by kernel() for the test harness


def _build():
    import concourse.bacc as bacc
    import concourse.mybir as mybir
    import concourse.tile as tile

    f32 = mybir.dt.float32
    b16 = mybir.dt.bfloat16
    Exp = mybir.ActivationFunctionType.Exp

    nc = bacc.Bacc("TRN2", target_bir_lowering=False)

    hT = nc.dram_tensor("hT", [HID, S], b16, kind="ExternalInput")
    wqT = nc.dram_tensor("wqT", [HID, OC], b16, kind="ExternalInput")
    wkT = nc.dram_tensor("wkT", [HID, OC], b16, kind="ExternalInput")
    wvT = nc.dram_tensor("wvT", [HID, OC], b16, kind="ExternalInput")
    woT = nc.dram_tensor("woT", [OC, HID], b16, kind="ExternalInput")
    cc = nc.dram_tensor("cc", [DH, S], b16, kind="ExternalInput")
    ssw = nc.dram_tensor("ssw", [DH, S], b16, kind="ExternalInput")
    ones = nc.dram_tensor("ones", [128, 128], b16, kind="ExternalInput")
    out_p = nc.dram_tensor("out_p", [S, HID], f32, kind="ExternalOutput")

    hT_r = hT.rearrange("(k p) t -> p k t", p=128)       # [128, 16, S]
    wqT_r = wqT.rearrange("(k p) o -> p k o", p=128)     # [128, 16, OC]
    wkT_r = wkT.rearrange("(k p) o -> p k o", p=128)
    wvT_r = wvT.rearrange("(k p) o -> p k o", p=128)
    woT_r = woT.rearrange("(h p) n -> p h n", p=128)     # [128, 4, HID]

    with tile.TileContext(nc) as tc:
        with (
            tc.tile_pool(name="const", bufs=1) as constp,
            tc.tile_pool(name="qkv", bufs=1) as qkvp,
            tc.tile_pool(name="rope", bufs=3) as ropep,
            tc.tile_pool(name="pbuf", bufs=3) as pp,
            tc.tile_pool(name="small", bufs=2) as smallp,
        ):
            wq_sb = constp.tile([128, KT, OC], b16, tag="wq")
            wk_sb = constp.tile([128, KT, OC], b16, tag="wk")
            wv_sb = constp.tile([128, KT, OC], b16, tag="wv")
            cc_sb = constp.tile([128, S], b16, tag="cc")
            ssw_sb = constp.tile([128, S], b16, tag="ssw")
            ones_sb = constp.tile([128, 128], b16, tag="ones")
            wo_sb = constp.tile([128, HPC, HID], b16, tag="wo")

            qT_sb = qkvp.tile([128, HPC, S], b16, tag="qT")
            kT_sb = qkvp.tile([128, HPC, S], b16, tag="kT")
            v_sb = qkvp.tile([128, NKB, OC], b16, tag="v")

            # ---- phase 1: QKV projection (+ fused RoPE for q,k) ----
            with (
                tc.tile_pool(name="hbuf", bufs=2) as hpool,
                tc.tile_pool(name="ps1qk", bufs=4, space="PSUM") as ps1qk,
                tc.tile_pool(name="ps1v", bufs=2, space="PSUM") as ps1v,
            ):
                # DMA order matters for the startup ramp: chunk 0's q-chains
                # consume wq[kk] + h0[kk] incrementally, so interleave those
                # per-kk up front; wo is only needed in phase 3 so it loads
                # last.
                h0 = []
                for kk in range(KT):
                    nc.sync.dma_start(out=wq_sb[:, kk, :], in_=wqT_r[:, kk, :])
                    ht = hpool.tile([128, CH], b16, tag=f"hch{kk}")
                    nc.sync.dma_start(out=ht, in_=hT_r[:, kk, 0:CH])
                    h0.append(ht)
                nc.sync.dma_start(out=cc_sb, in_=cc[:, :])
                nc.sync.dma_start(out=ssw_sb, in_=ssw[:, :])
                for kk in range(KT):
                    nc.sync.dma_start(out=wk_sb[:, kk, :], in_=wkT_r[:, kk, :])
                for kg in range(4):
                    nc.sync.dma_start(
                        out=wv_sb[:, kg * 4 : (kg + 1) * 4, :],
                        in_=wvT_r[:, kg * 4 : (kg + 1) * 4, :],
                    )
                nc.sync.dma_start(out=ones_sb, in_=ones[:, :])
                for hl in range(HPC):
                    nc.sync.dma_start(out=wo_sb[:, hl, :], in_=woT_r[:, hl, :])

                for ci in range(S // CH):
                    soff = ci * CH
                    if ci == 0:
                        hch = h0
                    else:
                        hch = []
                        for kk in range(KT):
                            ht = hpool.tile([128, CH], b16, tag=f"hch{kk}")
                            nc.sync.dma_start(
                                out=ht, in_=hT_r[:, kk, soff : soff + CH]
                            )
                            hch.append(ht)
                    # 8 outputs: q then k for each of the 4 heads
                    for qk, (wsb, dst) in enumerate(
                        ((wq_sb, qT_sb), (wk_sb, kT_sb))
                    ):
                        for hl in range(HPC):
                            ps = ps1qk.tile([128, CH], f32, tag="ps_qk")
                            for kk in range(KT):
                                nc.tensor.matmul(
                                    ps,
                                    wsb[:, kk, hl * DH : (hl + 1) * DH],
                                    hch[kk],
                                    start=(kk == 0),
                                    stop=(kk == KT - 1),
                                )
                            # RoPE: out = raw*cc + blockswap(raw)*ssw_signed
                            raw = ropep.tile([128, CH], b16, tag="raw")
                            nc.scalar.copy(raw, ps)
                            tmp = ropep.tile([128, CH], b16, tag="tmp")
                            nc.vector.tensor_mul(
                                tmp[0:64, :],
                                raw[64:128, :],
                                ssw_sb[64:128, soff : soff + CH],
                            )
                            nc.vector.tensor_mul(
                                tmp[64:128, :],
                                raw[0:64, :],
                                ssw_sb[0:64, soff : soff + CH],
                            )
                            dslice = dst[:, hl, soff : soff + CH]
                            nc.vector.tensor_mul(
                                dslice, raw, cc_sb[:, soff : soff + CH]
                            )
                            nc.vector.tensor_add(dslice, dslice, tmp)
                    for tt in range(CH // 128):
                        psv = ps1v.tile([128, OC], f32, tag="ps_v")
                        for kk in range(KT):
                            nc.tensor.matmul(
                                psv,
                                hch[kk][:, tt * 128 : (tt + 1) * 128],
                                wv_sb[:, kk, :],
                                start=(kk == 0),
                                stop=(kk == KT - 1),
                            )
                        nc.scalar.copy(v_sb[:, ci * (CH // 128) + tt, :], psv)

            # ---- phase 2: attention per head ----
            outT_sb = qkvp.tile([128, HPC, S], b16, tag="outT")
            with (
                tc.tile_pool(name="ps2s", bufs=2, space="PSUM") as ps2s,
                tc.tile_pool(name="ps2od", bufs=2, space="PSUM") as ps2od,
            ):
                for hl in range(HPC):
                    for qci in range(S // QC):
                        q0 = qci * QC
                        qmv = qT_sb[:, hl, q0 : q0 + QC]
                        psO = ps2od.tile([128, QC], f32, tag="psO")
                        psD = ps2od.tile([128, QC], f32, tag="psD")
                        padd_prev = None
                        for kg in range(NKB // 2):
                            pss = ps2s.tile([128, 2 * QC], f32, tag="pss")
                            for j in range(2):
                                kt = kg * 2 + j
                                nc.tensor.matmul(
                                    pss[:, j * QC : (j + 1) * QC],
                                    kT_sb[:, hl, kt * 128 : (kt + 1) * 128],
                                    qmv,
                                    skip_group_check=True,
                                )
                            pe = pp.tile([128, 2 * QC], b16, tag="pexp")
                            nc.scalar.activation(pe, pss, Exp, scale=SCALE)
                            for j in range(2):
                                kt = kg * 2 + j
                                nc.tensor.matmul(
                                    psO,
                                    v_sb[:, kt, hl * DH : (hl + 1) * DH],
                                    pe[:, j * QC : (j + 1) * QC],
                                    start=(kt == 0),
                                    stop=(kt == NKB - 1),
                                    skip_group_check=True,
                                )
                            # denominator: bf16 tree-reduce pairs of exp
                            # tiles on DVE, one ones-matmul per 4 k-blocks
                            padd = pp.tile([128, QC], b16, tag="padd")
                            nc.vector.tensor_add(
                                padd, pe[:, 0:QC], pe[:, QC : 2 * QC]
                            )
                            if kg % 2 == 0:
                                padd_prev = padd
                            else:
                                pquad = pp.tile([128, QC], b16, tag="pquad")
                                nc.vector.tensor_add(pquad, padd_prev, padd)
                                nc.tensor.matmul(
                                    psD,
                                    ones_sb,
                                    pquad,
                                    start=(kg == 1),
                                    stop=(kg == NKB // 2 - 1),
                                    skip_group_check=True,
                                )
                        dsb = smallp.tile([128, QC], f32, tag="dsb")
                        nc.vector.tensor_copy(dsb, psD)
                        rd = smallp.tile([128, QC], f32, tag="rd")
                        nc.vector.reciprocal_approx_fast(out=rd, in_=dsb)
                        nc.vector.tensor_mul(
                            outT_sb[:, hl, q0 : q0 + QC], psO, rd
                        )

            # ---- phase 3: output projection (partial over this core's heads) ----
            with (
                tc.tile_pool(name="fout", bufs=4) as foutp,
                tc.tile_pool(name="ps3", bufs=4, space="PSUM") as ps3,
            ):
                for tt in range(S // 128):
                    for nh in range(HID // 512):
                        psF = ps3.tile([128, 512], f32, tag="psF")
                        for hl in range(HPC):
                            nc.tensor.matmul(
                                psF,
                                outT_sb[:, hl, tt * 128 : (tt + 1) * 128],
                                wo_sb[:, hl, nh * 512 : (nh + 1) * 512],
                                start=(hl == 0),
                                stop=(hl == HPC - 1),
                            )
                        fo = foutp.tile([128, 512], f32, tag="fo")
                        if nh % 2 == 0:
                            nc.scalar.copy(fo, psF)
                        else:
                            nc.vector.tensor_copy(fo, psF)
                        nc.sync.dma_start(
                            out=out_p[
                                tt * 128 : (tt + 1) * 128,
                                nh * 512 : (nh + 1) * 512,
                            ],
                            in_=fo,
                        )

    nc.compile()
    return nc


def _deint(idx128):
    """de-interleave a [128] index block: evens then odds."""
    return np.concatenate([idx128[0::2], idx128[1::2]])


def _prep_inputs(hidden_states, cos, sin, w_qkv, w_o):
    """Host-side shard/layout prep. Returns per-core input maps."""
    # cos/sin transposed, de-interleaved: rows 0:64 = dims 0,2,..126 and
    # 64:128 = dims 1,3,..127. cos rows are pairwise equal so both halves
    # match. ssw is the sign-folded sin, pre-block-swapped so the RoPE
    # cross-partition multiplies read input partitions at one base:
    #   out[0:64]  = raw[64:128] * ssw[64:128]   (= -sin * odd part)
    #   out[64:128]= raw[0:64]   * ssw[0:64]     (= +sin * even part)
    ccf = np.concatenate([cos.T[0::2, :], cos.T[1::2, :]], axis=0).astype(BF)
    ssf = np.concatenate([sin.T[1::2, :], -sin.T[0::2, :]], axis=0).astype(BF)
    ones = np.ones((128, 128), dtype=BF)

    hT_b = [
        np.ascontiguousarray(hidden_states[b].T).astype(BF) for b in range(B)
    ]

    in_maps = []
    for c in range(NC):
        b = c // GPB
        heads = [HPC * (c % GPB) + i for i in range(HPC)]
        qrows = np.concatenate([_deint(np.arange(h * DH, (h + 1) * DH)) for h in heads])
        krows = H * DH + qrows
        vrows = (
            np.concatenate([np.arange(h * DH, (h + 1) * DH) for h in heads])
            + 2 * H * DH
        )
        ocols = np.concatenate([np.arange(h * DH, (h + 1) * DH) for h in heads])
        in_maps.append(
            {
                "hT": hT_b[b],
                "wqT": np.ascontiguousarray(w_qkv[qrows, :].T).astype(BF),
                "wkT": np.ascontiguousarray(w_qkv[krows, :].T).astype(BF),
                "wvT": np.ascontiguousarray(w_qkv[vrows, :].T).astype(BF),
                "woT": np.ascontiguousarray(w_o[:, ocols].T).astype(BF),
                "cc": ccf,
                "ssw": ssf,
                "ones": ones,
            }
        )
    return in_maps


def kernel(hidden_states, cos, sin, w_qkv, w_o):
    global _exec_time_ns
    from concourse.bass_utils import run_bass_kernel_spmd

    hidden_states = np.asarray(hidden_states, dtype=np.float32)
    cos = np.asarray(cos, dtype=np.float32)
    sin = np.asarray(sin, dtype=np.float32)
    w_qkv = np.asarray(w_qkv, dtype=np.float32)
    w_o = np.asarray(w_o, dtype=np.float32)

    nc = _build()
    in_maps = _prep_inputs(hidden_states, cos, sin, w_qkv, w_o)
    res = run_bass_kernel_spmd(
        nc,
        in_maps,
        core_ids=list(range(NC)),
        trace=bool(int(os.environ.get("KERNEL_TRACE", "0"))),
    )
    _exec_time_ns = res.exec_time_ns

    out = np.empty((B, S, HID), dtype=np.float32)
    for b in range(B):
        acc = res.results[b * GPB]["out_p"].astype(np.float32).copy()
        for c in range(b * GPB + 1, (b + 1) * GPB):
            acc += res.results[c]["out_p"]
        out[b] = acc
    return out
